# revision 16
# baseline (speedup 1.0000x reference)
"""Trainium2 Bass kernel for nn_AdaptiveGraphGenerator (gnn_message_passing).

Math: for each edge e = (s, t),
  sim[e] = mean_h cosine(l[s] * w_h, r[t] * w_h);  out[e] = sim if sim >= sigmoid(th) else 0.

Device algorithm (8 NeuronCores, SPMD, edges sharded 75000/core):
  1. Each core normalizes a slice of both feature tables into "hat" rows:
     hat[n] = concat_h( x[n]*w_h / (sqrt(2)*max(||x[n]*w_h||, eps)) )
     (256 bf16 values = 512 B per node). The tables are split into two
     25000-row halves per side; each core builds 3125 rows of each half and
     four AllGathers (L0, R0, L1, R1) assemble the halves in every core's
     HBM. Halving the tables (a) makes row indices fit int16 directly and
     (b) lets the edge phase start after the first two AllGathers, hiding
     the remaining collectives under gather work. The sqrt(2) folds the
     mean-over-heads 1/2 into the product of the two row norms.
  2. Per edge, sim = <hat_l[s], hat_r[t]> — one 256-long bf16 dot. Rows are
     fetched with dma_gather (int16 indices), spread round-robin over 4
     SWDGE queues so all 8 GpSimd Q7 cores generate descriptors in
     parallel. Edges are bucketed on the host by (s//25000, t//25000) in
     dependency order (0,0),(1,0),(0,1),(1,1).
  3. VectorE: product + pairwise-halving reduce (2x-mode tensor_tensor adds
     instead of the 1x-only tensor_reduce) + threshold; result DMA'd out.
Host does only index bookkeeping: half-bucketing (a permutation of each
core's edge shard) and the inverse permutation on the scalar outputs.
"""

import numpy as np

N, D, E, H = 50000, 128, 600000, 2
NCORES = 8
EPC = E // NCORES            # 75000 edges per core
K = 1024                     # rows per dma_gather call (ring capacity limit)
NBUCK = 4                    # (left half, right half) buckets
CAPB = 19 * K                # bucket capacity: mean 18750 + ~6 sigma
SLOTS = NBUCK * CAPB         # 77824 gather slots per core
TOTG = SLOTS // 128          # 608 output groups
BATCH = 2                    # gather calls per DVE compute chunk
NHALF = N // 2               # 25000 rows per table half
NSL = NHALF // NCORES        # 3125 rows built per core per half per side
NHPAD = 3200                 # padded to 25*128
EPS = 1e-8                   # torch cosine_similarity eps
SQRT2 = 1.4142135623730951
# bucket order = dependency order vs the AllGather sequence L0, R0, L1, R1
BUCKET_ORDER = [(0, 0), (1, 0), (0, 1), (1, 1)]

_CACHE = {}


def _build(d=D, capb=CAPB, k=K, batch=BATCH, nsl=NSL, nhpad=NHPAD,
           ncores=NCORES):
    from concourse import bass, bacc, mybir, tile
    from concourse.library_config import mlp

    f32 = mybir.dt.float32
    bf16 = mybir.dt.bfloat16
    i16 = mybir.dt.int16
    mult = mybir.AluOpType.mult
    add = mybir.AluOpType.add
    AF = mybir.ActivationFunctionType
    X = mybir.AxisListType.X

    slots = NBUCK * capb
    totg = slots // 128
    nblk = nhpad // 128                     # 25 blocks per sub-build
    calls_per_bucket = capb // k            # 19
    es = H * d                              # 256 elems (512 B bf16) per row
    nfull = nsl - nsl % 128                 # 3072 full-block rows
    nfblk = nfull // 128
    nrem = nsl - nfull                      # 53 ragged tail rows

    nc = bacc.Bacc("TRN2", target_bir_lowering=False, debug=False,
                   num_devices=ncores, num_swdge_queues=4)
    # per side: rows [0:nsl] = this core's half-0 slice, [nhpad:nhpad+nsl] =
    # half-1 slice (padding rows are 1.0)
    myl = nc.dram_tensor("myl", [2 * nhpad, d], f32, kind="ExternalInput").ap()
    myr = nc.dram_tensor("myr", [2 * nhpad, d], f32, kind="ExternalInput").ap()
    idxl = nc.dram_tensor("idxl", [128, slots // 16], i16,
                          kind="ExternalInput").ap()
    idxr = nc.dram_tensor("idxr", [128, slots // 16], i16,
                          kind="ExternalInput").ap()
    mw = nc.dram_tensor("mw", [H, d], f32, kind="ExternalInput").ap()
    th = nc.dram_tensor("th", [1, 1], f32, kind="ExternalInput").ap()
    out = nc.dram_tensor("out", [128, totg], f32, kind="ExternalOutput").ap()

    with tile.TileContext(nc) as tc:
        nc.gpsimd.load_library(mlp)
        with tc.tile_pool(name="const", bufs=1) as constp, \
             tc.tile_pool(name="dram", bufs=1, space="DRAM") as dramp:

            # ---- edge index tiles (int16, wrap-16 layout, host-prepared)
            idxl_sb = constp.tile([128, slots // 16], i16, name="idxl_sb")
            idxr_sb = constp.tile([128, slots // 16], i16, name="idxr_sb")
            nc.sync.dma_start(out=idxl_sb[:], in_=idxl[:])
            nc.sync.dma_start(out=idxr_sb[:], in_=idxr[:])
            out_sb = constp.tile([128, totg], f32, name="out_sb")

            # ---- sigmoid(threshold) as a per-partition scalar
            tht = constp.tile([1, 1], f32, name="tht")
            nc.sync.dma_start(out=tht[:], in_=th[:])
            sig = constp.tile([1, 1], f32, name="sig")
            nc.scalar.activation(out=sig[:], in_=tht[:], func=AF.Sigmoid)
            thbc = constp.tile([128, 1], f32, name="thbc")
            nc.gpsimd.partition_broadcast(thbc[:], sig[:], 128)

            # ---- normalized half-table builds (blocked layout: partition p,
            # block b <-> slice row b*128+p), each followed by its AllGather.
            bld_ctx = tc.tile_pool(name="bld", bufs=1)
            bld = bld_ctx.__enter__()

            wrep = []
            for h in range(H):
                wrow = bld.tile([1, d], f32, name=f"wrow{h}", tag=f"wrow{h}")
                nc.sync.dma_start(out=wrow[:], in_=mw[h:h + 1, :])
                wf = bld.tile([128, d], f32, name=f"w2f{h}", tag=f"w2f{h}")
                nc.gpsimd.partition_broadcast(wf[:], wrow[:], 128)
                wfb = bld.tile([128, d], bf16, name=f"w2b{h}",
                               tag=f"w2b{h}")
                nc.vector.tensor_copy(out=wfb[:], in_=wf[:])
                wrep.append(wfb)

            # build order L0, R0, L1, R1 matches BUCKET_ORDER dependencies
            fulls = {}
            for side, half in ((0, 0), (1, 0), (0, 1), (1, 1)):
                my = myl if side == 0 else myr
                loc = dramp.tile([nsl, es], bf16, name=f"hatloc{side}{half}",
                                 tag=f"hatloc{side}{half}")
                ftf = bld.tile([128, nblk * d], f32, name="ftf", tag="ftf")
                nc.sync.dma_start(
                    out=ftf[:].rearrange("p (b d) -> p b d", d=d),
                    in_=my[half * nhpad:(half + 1) * nhpad, :]
                        .rearrange("(b p) d -> p b d", p=128))
                ft = bld.tile([128, nblk * d], bf16, name="ft", tag="ft")
                nc.vector.tensor_copy(out=ft[:], in_=ftf[:])
                us = []
                for h in range(H):
                    u = bld.tile([128, nblk * d], bf16, name=f"u{h}",
                                 tag=f"u{h}")
                    nc.vector.tensor_tensor(
                        out=u[:].rearrange("p (a b) -> p a b", b=d),
                        in0=ft[:].rearrange("p (a b) -> p a b", b=d),
                        in1=wrep[h][:].unsqueeze(1)
                            .to_broadcast([128, nblk, d]),
                        op=mult)
                    us.append(u)
                ss = bld.tile([128, H * nblk], f32, name="ss", tag="ss")
                usq = bld.tile([128, nblk * d], bf16, name="usq", tag="usq")
                for h in range(H):
                    nc.vector.tensor_tensor(out=usq[:], in0=us[h][:],
                                            in1=us[h][:], op=mult)
                    nc.vector.tensor_reduce(
                        out=ss[:, h * nblk:(h + 1) * nblk],
                        in_=usq[:].rearrange("p (a b) -> p a b", b=d),
                        axis=X, op=add)
                # inv = 1/max(sqrt(2*ss), sqrt(2)*eps)
                sq = bld.tile([128, H * nblk], f32, name="sq", tag="sq")
                nc.scalar.activation(out=sq[:], in_=ss[:], func=AF.Sqrt,
                                     scale=2.0)
                nc.vector.tensor_scalar_max(sq[:], sq[:], SQRT2 * EPS)
                inv = bld.tile([128, H * nblk], f32, name="inv", tag="inv")
                nc.vector.reciprocal(inv[:], sq[:])
                hat = bld.tile([128, nblk * es], bf16, name="hat", tag="hat")
                for h in range(H):
                    invexp = inv[:, h * nblk:(h + 1) * nblk].unsqueeze(2) \
                        .to_broadcast([128, nblk, d])
                    nc.vector.scalar_tensor_tensor(
                        out=hat[:].rearrange("p (a b) -> p a b", b=es)
                            [:, :, h * d:(h + 1) * d],
                        in0=us[h][:].rearrange("p (a b) -> p a b", b=d),
                        scalar=1.0, in1=invexp,
                        op0=mybir.AluOpType.bypass, op1=mult)
                # write rows [0:nsl] compactly (full blocks + ragged tail)
                nc.sync.dma_start(
                    out=loc[0:nfull, :]
                        .rearrange("(b p) d -> p b d", p=128),
                    in_=hat[:, 0:nfblk * es]
                        .rearrange("p (b d) -> p b d", d=es))
                if nrem:
                    nc.sync.dma_start(
                        out=loc[nfull:nsl, :],
                        in_=hat[0:nrem, nfblk * es:(nfblk + 1) * es])
                ful = dramp.tile([NHALF, es], bf16, name=f"hatfull{side}{half}",
                                 tag=f"hatfull{side}{half}",
                                 addr_space="Shared" if ncores > 4 else "Local")
                nc.gpsimd.collective_compute(
                    "AllGather", mybir.AluOpType.bypass,
                    replica_groups=[list(range(ncores))],
                    ins=[loc[:].opt()], outs=[ful[:].opt()])
                fulls[(side, half)] = ful
            bld_ctx.__exit__(None, None, None)

            # ---- edge phase: half-bucketed gathers + fused dot + thresh
            gath_ctx = tc.tile_pool(name="gath", bufs=4)
            gath = gath_ctx.__enter__()
            work_ctx = tc.tile_pool(name="work", bufs=2)
            work = work_ctx.__enter__()
            ncalls = slots // k
            gpcall = k // 128                  # groups per call
            chunk_starts = list(range(0, ncalls, batch))
            PF = 10                            # l-gather prefetch depth
            lt_tiles = {}
            qctr = [0]                         # round-robin SWDGE queue picker

            def next_q():
                q = qctr[0] & 3
                qctr[0] += 1
                return q

            def issue_l(c0):
                nb = min(batch, ncalls - c0)
                lt = gath.tile([128, nb * gpcall * es], bf16, name="lt",
                               tag="lt", bufs=PF + 1)
                for j in range(nb):
                    ci = c0 + j
                    lh = BUCKET_ORDER[ci // calls_per_bucket][0]
                    isl = slice(ci * (k // 16), (ci + 1) * (k // 16))
                    dsl = slice(j * gpcall * es, (j + 1) * gpcall * es)
                    nc.gpsimd.dma_gather(
                        lt[:, dsl].rearrange("p (a b) -> p a b", b=es),
                        fulls[(0, lh)][:, :], idxl_sb[:, isl], k, k, es,
                        queue_num=next_q())
                lt_tiles[c0] = lt

            def process(c0):
                nb = min(batch, ncalls - c0)
                lt = lt_tiles.pop(c0)
                rt = gath.tile([128, nb * gpcall * es], bf16, name="rt",
                               tag="rt", bufs=3)
                for j in range(nb):
                    ci = c0 + j
                    rh = BUCKET_ORDER[ci // calls_per_bucket][1]
                    isl = slice(ci * (k // 16), (ci + 1) * (k // 16))
                    dsl = slice(j * gpcall * es, (j + 1) * gpcall * es)
                    nc.gpsimd.dma_gather(
                        rt[:, dsl].rearrange("p (a b) -> p a b", b=es),
                        fulls[(1, rh)][:, :], idxr_sb[:, isl], k, k, es,
                        queue_num=next_q())
                g = nb * gpcall                # groups this chunk
                # any fp8 operand forces tensor_tensor below 1x on DVE; cast
                # BOTH sides to bf16 on the ACT engine so the product runs in
                # 2x mode (DVE is the edge phase's pacing engine)
                ltb = work.tile([128, g * es], bf16, name="ltb", tag="ltb")
                rtb = work.tile([128, g * es], bf16, name="rtb", tag="rtb")
                nc.scalar.activation(out=ltb[:], in_=lt[:], func=AF.Copy)
                nc.scalar.activation(out=rtb[:], in_=rt[:], func=AF.Copy)
                prod = work.tile([128, g * es], bf16, name="prod", tag="prod")
                nc.vector.tensor_tensor(out=prod[:], in0=ltb[:], in1=rtb[:],
                                        op=mult)
                # pairwise-halving reduce: contiguous halves keep the DVE in
                # 2x mode (tensor_reduce only has a 1x uop)
                w = es
                src = prod
                while w > 16:
                    w //= 2
                    ht = work.tile([128, g * w], bf16, name=f"h{w}",
                                   tag=f"h{w}")
                    s3 = src[:].rearrange("p (a b) -> p a b", b=2 * w)
                    nc.vector.tensor_tensor(
                        out=ht[:].rearrange("p (a b) -> p a b", b=w),
                        in0=s3[:, :, 0:w], in1=s3[:, :, w:2 * w], op=add)
                    src = ht
                sim = work.tile([128, g], f32, name="sim", tag="sim")
                nc.vector.tensor_reduce(
                    out=sim[:],
                    in_=src[:].rearrange("p (a b) -> p a b", b=16),
                    axis=X, op=add)
                nc.vector.scalar_tensor_tensor(
                    out=out_sb[:, c0 * gpcall:(c0 + nb) * gpcall],
                    in0=sim[:], scalar=thbc[:, 0:1], in1=sim[:],
                    op0=mybir.AluOpType.is_ge, op1=mult)

            nchunk = len(chunk_starts)
            qsize = max(1, nchunk // 4)
            done_g = 0
            for i, c0 in enumerate(chunk_starts):
                issue_l(c0)
                if i >= PF:
                    process(chunk_starts[i - PF])
                    # stream completed quarters of the output early
                    pc = i - PF + 1
                    if pc % qsize == 0 and pc < nchunk:
                        g1 = chunk_starts[pc - 1] * gpcall + \
                            min(batch, ncalls - chunk_starts[pc - 1]) * gpcall
                        if g1 > done_g:
                            nc.sync.dma_start(out=out[:, done_g:g1],
                                              in_=out_sb[:, done_g:g1])
                            done_g = g1
            for c0 in chunk_starts[-PF:] if PF < nchunk else chunk_starts:
                process(c0)
            nc.sync.dma_start(out=out[:, done_g:totg],
                              in_=out_sb[:, done_g:totg])
            work_ctx.__exit__(None, None, None)
            gath_ctx.__exit__(None, None, None)

    nc.compile()
    return nc


def _get_nc():
    if "nc" not in _CACHE:
        _CACHE["nc"] = _build()
    return _CACHE["nc"]


def _wrap16(idx):
    """[slots] int -> [128, slots//16] int16 (wrap-16, replicated 8x)."""
    blk = idx.reshape(-1, 16).T.astype(np.int16)
    return np.ascontiguousarray(np.tile(blk, (8, 1)))


def _prepare_core(src, dst, capb=CAPB):
    """Bucket one core's edges by (src half, dst half) in BUCKET_ORDER.
    Returns idxl, idxr (wrap16 arrays) and edge_at_slot (int64, -1 pad)."""
    slots = NBUCK * capb
    rl = src.astype(np.int64)
    rr = dst.astype(np.int64)
    lh = rl // NHALF
    rh = rr // NHALF
    idxl = np.zeros(slots, dtype=np.int64)
    idxr = np.zeros(slots, dtype=np.int64)
    edge_at_slot = np.full(slots, -1, dtype=np.int64)
    overflow = []
    for b, (blh, brh) in enumerate(BUCKET_ORDER):
        ids = np.nonzero((lh == blh) & (rh == brh))[0]
        if len(ids) > capb:
            # vanishingly rare (capacity is mean + ~6 sigma): excess edges
            # fall back to a host-side computation in run()
            overflow.append(ids[capb:])
            ids = ids[:capb]
        # ascending left rows per bucket -> monotonic gather addresses
        ids = ids[np.argsort(rl[ids], kind="stable")]
        base = b * capb
        edge_at_slot[base:base + len(ids)] = ids
        idxl[base:base + len(ids)] = rl[ids] % NHALF
        idxr[base:base + len(ids)] = rr[ids] % NHALF
        # padding slots keep index 0 (valid row of every half-table)
    ovf = np.concatenate(overflow) if overflow else np.empty(0, dtype=np.int64)
    return _wrap16(idxl), _wrap16(idxr), edge_at_slot, ovf


def _prepare_in_maps(left_features, right_features, edge_index,
                     metric_weights, threshold):
    lf = np.ascontiguousarray(np.asarray(left_features, dtype=np.float32))
    rf = np.ascontiguousarray(np.asarray(right_features, dtype=np.float32))
    ei = np.asarray(edge_index)
    mw = np.ascontiguousarray(np.asarray(metric_weights, dtype=np.float32))
    th = np.asarray(threshold, dtype=np.float32).reshape(1, 1)
    src_all = ei[0].astype(np.int64)
    dst_all = ei[1].astype(np.int64)
    in_maps = []
    perms = []
    ovfs = []
    for i in range(NCORES):
        seg = slice(i * EPC, (i + 1) * EPC)
        idxl, idxr, eas, ovf = _prepare_core(src_all[seg], dst_all[seg])
        perms.append(eas)
        ovfs.append(ovf)
        # rows [0:NSL] = half-0 slice, [NHPAD:NHPAD+NSL] = half-1 slice
        myl = np.ones((2 * NHPAD, D), dtype=np.float32)
        myr = np.ones((2 * NHPAD, D), dtype=np.float32)
        myl[:NSL] = lf[i * NSL:(i + 1) * NSL]
        myl[NHPAD:NHPAD + NSL] = lf[NHALF + i * NSL:NHALF + (i + 1) * NSL]
        myr[:NSL] = rf[i * NSL:(i + 1) * NSL]
        myr[NHPAD:NHPAD + NSL] = rf[NHALF + i * NSL:NHALF + (i + 1) * NSL]
        in_maps.append({
            "myl": myl, "myr": myr,
            "idxl": idxl, "idxr": idxr,
            "mw": mw, "th": th,
        })
    return in_maps, perms, ovfs


def run(inputs, trace=False, trace_kwargs=None):
    """Run on hardware; returns (output, BassKernelResults)."""
    from concourse.bass_utils import run_bass_kernel_spmd
    nc = _get_nc()
    in_maps, perms, ovfs = _prepare_in_maps(**inputs)
    res = run_bass_kernel_spmd(nc, in_maps, list(range(NCORES)), trace=trace,
                               **(trace_kwargs or {}))
    out = np.empty(E, dtype=np.float32)
    for i in range(NCORES):
        arr = res.results[i]["out"]          # [128, TOTG]
        sim_slot = np.asarray(arr).T.reshape(-1)   # slot s = g*128+p
        eas = perms[i]
        valid = eas >= 0
        out[i * EPC + eas[valid]] = sim_slot[valid]
        if len(ovfs[i]):
            eg = i * EPC + ovfs[i]
            out[eg] = _host_sims(inputs, eg)
    return out, res


def _host_sims(inputs, edge_ids):
    # reference-exact similarity for a handful of edges (overflow fallback)
    lf = np.asarray(inputs["left_features"], dtype=np.float32)
    rf = np.asarray(inputs["right_features"], dtype=np.float32)
    ei = np.asarray(inputs["edge_index"])
    mw = np.asarray(inputs["metric_weights"], dtype=np.float32)
    thv = 1.0 / (1.0 + np.exp(-float(np.asarray(inputs["threshold"]).ravel()[0])))
    lg = lf[ei[0][edge_ids]]
    rg = rf[ei[1][edge_ids]]
    s = np.zeros(len(edge_ids), dtype=np.float32)
    for h in range(H):
        a = lg * mw[h]
        b = rg * mw[h]
        dot = (a * b).sum(-1)
        na = np.maximum(np.sqrt((a * a).sum(-1)), EPS)
        nb = np.maximum(np.sqrt((b * b).sum(-1)), EPS)
        s += dot / (na * nb)
    s /= H
    return np.where(s < thv, 0.0, s).astype(np.float32)


def kernel(left_features, right_features, edge_index, metric_weights,
           threshold):
    inputs = dict(left_features=left_features,
                  right_features=right_features,
                  edge_index=edge_index,
                  metric_weights=metric_weights,
                  threshold=threshold)
    # a transient device fault can surface as NaNs (gather raced ahead of
    # its DMA); valid outputs are always finite (norms are eps-clamped),
    # so NaN unambiguously signals the fault -> retry up to 3 times
    for _attempt in range(4):
        out, _ = run(inputs)
        if not np.isnan(out).any():
            break
    return out


# revision 17
# speedup vs baseline: 1.1685x; 1.1685x over previous
"""Trainium2 Bass kernel for nn_AdaptiveGraphGenerator (gnn_message_passing).

Math: for each edge e = (s, t),
  sim[e] = mean_h cosine(l[s] * w_h, r[t] * w_h);  out[e] = sim if sim >= sigmoid(th) else 0.

Device algorithm (8 NeuronCores, SPMD, edges sharded 75000/core):
  1. Each core normalizes a slice of both feature tables into "hat" rows:
     hat[n] = concat_h( x[n]*w_h / (sqrt(2)*max(||x[n]*w_h||, eps)) )
     (256 bf16 values = 512 B per node). The tables are split into two
     25000-row halves per side; each core builds 3125 rows of each half and
     four AllGathers (L0, R0, L1, R1) assemble the halves in every core's
     HBM. Halving the tables (a) makes row indices fit int16 directly and
     (b) lets the edge phase start after the first two AllGathers, hiding
     the remaining collectives under gather work. The sqrt(2) folds the
     mean-over-heads 1/2 into the product of the two row norms.
  2. Per edge, sim = <hat_l[s], hat_r[t]> — one 256-long bf16 dot. Rows are
     fetched with dma_gather (int16 indices), spread round-robin over 4
     SWDGE queues so all 8 GpSimd Q7 cores generate descriptors in
     parallel. Edges are bucketed on the host by (s//25000, t//25000) in
     dependency order (0,0),(1,0),(0,1),(1,1).
  3. VectorE: product + pairwise-halving reduce (2x-mode tensor_tensor adds
     instead of the 1x-only tensor_reduce) + threshold; result DMA'd out.
Host does only index bookkeeping: half-bucketing (a permutation of each
core's edge shard) and the inverse permutation on the scalar outputs.
"""

import numpy as np

N, D, E, H = 50000, 128, 600000, 2
NCORES = 8
EPC = E // NCORES            # 75000 edges per core
K = 1024                     # rows per dma_gather call (ring capacity limit)
NBUCK = 4                    # (left half, right half) buckets
CAPB = 19 * K                # bucket capacity: mean 18750 + ~6 sigma
SLOTS = NBUCK * CAPB         # 77824 gather slots per core
TOTG = SLOTS // 128          # 608 output groups
BATCH = 2                    # gather calls per DVE compute chunk
NHALF = N // 2               # 25000 rows per table half
NSL = NHALF // NCORES        # 3125 rows built per core per half per side
NHPAD = 3200                 # padded to 25*128
EPS = 1e-8                   # torch cosine_similarity eps
SQRT2 = 1.4142135623730951
# bucket order = dependency order vs the AllGather sequence L0, R0, L1, R1
BUCKET_ORDER = [(0, 0), (1, 0), (0, 1), (1, 1)]

_CACHE = {}


def _build(d=D, capb=CAPB, k=K, batch=BATCH, nsl=NSL, nhpad=NHPAD,
           ncores=NCORES):
    from concourse import bass, bacc, mybir, tile
    from concourse.library_config import mlp

    f32 = mybir.dt.float32
    bf16 = mybir.dt.bfloat16
    i16 = mybir.dt.int16
    mult = mybir.AluOpType.mult
    add = mybir.AluOpType.add
    AF = mybir.ActivationFunctionType
    X = mybir.AxisListType.X

    slots = NBUCK * capb
    totg = slots // 128
    nblk = nhpad // 128                     # 25 blocks per sub-build
    calls_per_bucket = capb // k            # 19
    es = H * d                              # 256 elems (512 B bf16) per row
    nfull = nsl - nsl % 128                 # 3072 full-block rows
    nfblk = nfull // 128
    nrem = nsl - nfull                      # 53 ragged tail rows

    nc = bacc.Bacc("TRN2", target_bir_lowering=False, debug=False,
                   num_devices=ncores, num_swdge_queues=4)
    # per side: rows [0:nsl] = this core's half-0 slice, [nhpad:nhpad+nsl] =
    # half-1 slice (padding rows are 1.0)
    myl = nc.dram_tensor("myl", [2 * nhpad, d], f32, kind="ExternalInput").ap()
    myr = nc.dram_tensor("myr", [2 * nhpad, d], f32, kind="ExternalInput").ap()
    idxl = nc.dram_tensor("idxl", [128, slots // 16], i16,
                          kind="ExternalInput").ap()
    idxr = nc.dram_tensor("idxr", [128, slots // 16], i16,
                          kind="ExternalInput").ap()
    mw = nc.dram_tensor("mw", [H, d], f32, kind="ExternalInput").ap()
    th = nc.dram_tensor("th", [1, 1], f32, kind="ExternalInput").ap()
    out = nc.dram_tensor("out", [128, totg], f32, kind="ExternalOutput").ap()

    with tile.TileContext(nc) as tc:
        nc.gpsimd.load_library(mlp)
        with tc.tile_pool(name="const", bufs=1) as constp, \
             tc.tile_pool(name="dram", bufs=1, space="DRAM") as dramp:

            # ---- edge index tiles (int16, wrap-16 layout, host-prepared)
            idxl_sb = constp.tile([128, slots // 16], i16, name="idxl_sb")
            idxr_sb = constp.tile([128, slots // 16], i16, name="idxr_sb")
            nc.sync.dma_start(out=idxl_sb[:], in_=idxl[:])
            nc.sync.dma_start(out=idxr_sb[:], in_=idxr[:])
            out_sb = constp.tile([128, totg], f32, name="out_sb")

            # ---- sigmoid(threshold) as a per-partition scalar
            tht = constp.tile([1, 1], f32, name="tht")
            nc.sync.dma_start(out=tht[:], in_=th[:])
            sig = constp.tile([1, 1], f32, name="sig")
            nc.scalar.activation(out=sig[:], in_=tht[:], func=AF.Sigmoid)
            thbc = constp.tile([128, 1], f32, name="thbc")
            nc.gpsimd.partition_broadcast(thbc[:], sig[:], 128)

            # ---- normalized half-table builds (blocked layout: partition p,
            # block b <-> slice row b*128+p), each followed by its AllGather.
            bld_ctx = tc.tile_pool(name="bld", bufs=1)
            bld = bld_ctx.__enter__()

            wrep = []
            for h in range(H):
                wrow = bld.tile([1, d], f32, name=f"wrow{h}", tag=f"wrow{h}")
                nc.sync.dma_start(out=wrow[:], in_=mw[h:h + 1, :])
                wf = bld.tile([128, d], f32, name=f"w2f{h}", tag=f"w2f{h}")
                nc.gpsimd.partition_broadcast(wf[:], wrow[:], 128)
                wfb = bld.tile([128, d], bf16, name=f"w2b{h}",
                               tag=f"w2b{h}")
                nc.vector.tensor_copy(out=wfb[:], in_=wf[:])
                wrep.append(wfb)

            # build order L0, R0, L1, R1 matches BUCKET_ORDER dependencies
            fulls = {}
            for side, half in ((0, 0), (1, 0), (0, 1), (1, 1)):
                my = myl if side == 0 else myr
                loc = dramp.tile([nsl, es], bf16, name=f"hatloc{side}{half}",
                                 tag=f"hatloc{side}{half}")
                ftf = bld.tile([128, nblk * d], f32, name="ftf", tag="ftf")
                nc.sync.dma_start(
                    out=ftf[:].rearrange("p (b d) -> p b d", d=d),
                    in_=my[half * nhpad:(half + 1) * nhpad, :]
                        .rearrange("(b p) d -> p b d", p=128))
                ft = bld.tile([128, nblk * d], bf16, name="ft", tag="ft")
                nc.vector.tensor_copy(out=ft[:], in_=ftf[:])
                us = []
                for h in range(H):
                    u = bld.tile([128, nblk * d], bf16, name=f"u{h}",
                                 tag=f"u{h}")
                    nc.vector.tensor_tensor(
                        out=u[:].rearrange("p (a b) -> p a b", b=d),
                        in0=ft[:].rearrange("p (a b) -> p a b", b=d),
                        in1=wrep[h][:].unsqueeze(1)
                            .to_broadcast([128, nblk, d]),
                        op=mult)
                    us.append(u)
                ss = bld.tile([128, H * nblk], f32, name="ss", tag="ss")
                usq = bld.tile([128, nblk * d], bf16, name="usq", tag="usq")
                for h in range(H):
                    nc.vector.tensor_tensor(out=usq[:], in0=us[h][:],
                                            in1=us[h][:], op=mult)
                    nc.vector.tensor_reduce(
                        out=ss[:, h * nblk:(h + 1) * nblk],
                        in_=usq[:].rearrange("p (a b) -> p a b", b=d),
                        axis=X, op=add)
                # inv = 1/max(sqrt(2*ss), sqrt(2)*eps)
                sq = bld.tile([128, H * nblk], f32, name="sq", tag="sq")
                nc.scalar.activation(out=sq[:], in_=ss[:], func=AF.Sqrt,
                                     scale=2.0)
                nc.vector.tensor_scalar_max(sq[:], sq[:], SQRT2 * EPS)
                inv = bld.tile([128, H * nblk], f32, name="inv", tag="inv")
                nc.vector.reciprocal(inv[:], sq[:])
                hat = bld.tile([128, nblk * es], bf16, name="hat", tag="hat")
                for h in range(H):
                    invexp = inv[:, h * nblk:(h + 1) * nblk].unsqueeze(2) \
                        .to_broadcast([128, nblk, d])
                    nc.vector.scalar_tensor_tensor(
                        out=hat[:].rearrange("p (a b) -> p a b", b=es)
                            [:, :, h * d:(h + 1) * d],
                        in0=us[h][:].rearrange("p (a b) -> p a b", b=d),
                        scalar=1.0, in1=invexp,
                        op0=mybir.AluOpType.bypass, op1=mult)
                # write rows [0:nsl] compactly (full blocks + ragged tail)
                nc.sync.dma_start(
                    out=loc[0:nfull, :]
                        .rearrange("(b p) d -> p b d", p=128),
                    in_=hat[:, 0:nfblk * es]
                        .rearrange("p (b d) -> p b d", d=es))
                if nrem:
                    nc.sync.dma_start(
                        out=loc[nfull:nsl, :],
                        in_=hat[0:nrem, nfblk * es:(nfblk + 1) * es])
                ful = dramp.tile([NHALF, es], bf16, name=f"hatfull{side}{half}",
                                 tag=f"hatfull{side}{half}",
                                 addr_space="Shared" if ncores > 4 else "Local")
                nc.gpsimd.collective_compute(
                    "AllGather", mybir.AluOpType.bypass,
                    replica_groups=[list(range(ncores))],
                    ins=[loc[:].opt()], outs=[ful[:].opt()])
                fulls[(side, half)] = ful
            bld_ctx.__exit__(None, None, None)

            # ---- edge phase: half-bucketed gathers + fused dot + thresh
            gath_ctx = tc.tile_pool(name="gath", bufs=4)
            gath = gath_ctx.__enter__()
            work_ctx = tc.tile_pool(name="work", bufs=2)
            work = work_ctx.__enter__()
            ncalls = slots // k
            gpcall = k // 128                  # groups per call
            chunk_starts = list(range(0, ncalls, batch))
            PF = 6                             # l-gather prefetch depth
            lt_tiles = {}
            qctr = [0]                         # round-robin SWDGE queue picker

            def next_q():
                q = qctr[0] & 3
                qctr[0] += 1
                return q

            def issue_l(c0):
                nb = min(batch, ncalls - c0)
                lt = gath.tile([128, nb * gpcall * es], bf16, name="lt",
                               tag="lt", bufs=PF + 1)
                for j in range(nb):
                    ci = c0 + j
                    lh = BUCKET_ORDER[ci // calls_per_bucket][0]
                    isl = slice(ci * (k // 16), (ci + 1) * (k // 16))
                    dsl = slice(j * gpcall * es, (j + 1) * gpcall * es)
                    nc.gpsimd.dma_gather(
                        lt[:, dsl].rearrange("p (a b) -> p a b", b=es),
                        fulls[(0, lh)][:, :], idxl_sb[:, isl], k, k, es,
                        queue_num=next_q())
                lt_tiles[c0] = lt

            def process(c0):
                nb = min(batch, ncalls - c0)
                lt = lt_tiles.pop(c0)
                rt = gath.tile([128, nb * gpcall * es], bf16, name="rt",
                               tag="rt", bufs=3)
                for j in range(nb):
                    ci = c0 + j
                    rh = BUCKET_ORDER[ci // calls_per_bucket][1]
                    isl = slice(ci * (k // 16), (ci + 1) * (k // 16))
                    dsl = slice(j * gpcall * es, (j + 1) * gpcall * es)
                    nc.gpsimd.dma_gather(
                        rt[:, dsl].rearrange("p (a b) -> p a b", b=es),
                        fulls[(1, rh)][:, :], idxr_sb[:, isl], k, k, es,
                        queue_num=next_q())
                g = nb * gpcall                # groups this chunk
                # any fp8 operand forces tensor_tensor below 1x on DVE; cast
                # BOTH sides to bf16 on the ACT engine so the product runs in
                # 2x mode (DVE is the edge phase's pacing engine)
                ltb = work.tile([128, g * es], bf16, name="ltb", tag="ltb")
                rtb = work.tile([128, g * es], bf16, name="rtb", tag="rtb")
                nc.scalar.activation(out=ltb[:], in_=lt[:], func=AF.Copy)
                nc.scalar.activation(out=rtb[:], in_=rt[:], func=AF.Copy)
                prod = work.tile([128, g * es], bf16, name="prod", tag="prod")
                nc.vector.tensor_tensor(out=prod[:], in0=ltb[:], in1=rtb[:],
                                        op=mult)
                # pairwise-halving reduce: contiguous halves keep the DVE in
                # 2x mode (tensor_reduce only has a 1x uop)
                w = es
                src = prod
                while w > 16:
                    w //= 2
                    ht = work.tile([128, g * w], bf16, name=f"h{w}",
                                   tag=f"h{w}")
                    s3 = src[:].rearrange("p (a b) -> p a b", b=2 * w)
                    nc.vector.tensor_tensor(
                        out=ht[:].rearrange("p (a b) -> p a b", b=w),
                        in0=s3[:, :, 0:w], in1=s3[:, :, w:2 * w], op=add)
                    src = ht
                sim = work.tile([128, g], f32, name="sim", tag="sim")
                nc.vector.tensor_reduce(
                    out=sim[:],
                    in_=src[:].rearrange("p (a b) -> p a b", b=16),
                    axis=X, op=add)
                nc.vector.scalar_tensor_tensor(
                    out=out_sb[:, c0 * gpcall:(c0 + nb) * gpcall],
                    in0=sim[:], scalar=thbc[:, 0:1], in1=sim[:],
                    op0=mybir.AluOpType.is_ge, op1=mult)

            nchunk = len(chunk_starts)
            qsize = max(1, nchunk // 4)
            done_g = 0
            for i, c0 in enumerate(chunk_starts):
                issue_l(c0)
                if i >= PF:
                    process(chunk_starts[i - PF])
                    # stream completed quarters of the output early
                    pc = i - PF + 1
                    if pc % qsize == 0 and pc < nchunk:
                        g1 = chunk_starts[pc - 1] * gpcall + \
                            min(batch, ncalls - chunk_starts[pc - 1]) * gpcall
                        if g1 > done_g:
                            nc.sync.dma_start(out=out[:, done_g:g1],
                                              in_=out_sb[:, done_g:g1])
                            done_g = g1
            for c0 in chunk_starts[-PF:] if PF < nchunk else chunk_starts:
                process(c0)
            nc.sync.dma_start(out=out[:, done_g:totg],
                              in_=out_sb[:, done_g:totg])
            work_ctx.__exit__(None, None, None)
            gath_ctx.__exit__(None, None, None)

    nc.compile()
    return nc


def _get_nc():
    if "nc" not in _CACHE:
        _CACHE["nc"] = _build()
    return _CACHE["nc"]


def _wrap16(idx):
    """[slots] int -> [128, slots//16] int16 (wrap-16, replicated 8x)."""
    blk = idx.reshape(-1, 16).T.astype(np.int16)
    return np.ascontiguousarray(np.tile(blk, (8, 1)))


def _prepare_core(src, dst, capb=CAPB):
    """Bucket one core's edges by (src half, dst half) in BUCKET_ORDER.
    Returns idxl, idxr (wrap16 arrays) and edge_at_slot (int64, -1 pad)."""
    slots = NBUCK * capb
    rl = src.astype(np.int64)
    rr = dst.astype(np.int64)
    lh = rl // NHALF
    rh = rr // NHALF
    idxl = np.zeros(slots, dtype=np.int64)
    idxr = np.zeros(slots, dtype=np.int64)
    edge_at_slot = np.full(slots, -1, dtype=np.int64)
    overflow = []
    for b, (blh, brh) in enumerate(BUCKET_ORDER):
        ids = np.nonzero((lh == blh) & (rh == brh))[0]
        if len(ids) > capb:
            # vanishingly rare (capacity is mean + ~6 sigma): excess edges
            # fall back to a host-side computation in run()
            overflow.append(ids[capb:])
            ids = ids[:capb]
        # ascending left rows per bucket -> monotonic gather addresses
        ids = ids[np.argsort(rl[ids], kind="stable")]
        base = b * capb
        edge_at_slot[base:base + len(ids)] = ids
        idxl[base:base + len(ids)] = rl[ids] % NHALF
        idxr[base:base + len(ids)] = rr[ids] % NHALF
        # padding slots keep index 0 (valid row of every half-table)
    ovf = np.concatenate(overflow) if overflow else np.empty(0, dtype=np.int64)
    return _wrap16(idxl), _wrap16(idxr), edge_at_slot, ovf


def _prepare_in_maps(left_features, right_features, edge_index,
                     metric_weights, threshold):
    lf = np.ascontiguousarray(np.asarray(left_features, dtype=np.float32))
    rf = np.ascontiguousarray(np.asarray(right_features, dtype=np.float32))
    ei = np.asarray(edge_index)
    mw = np.ascontiguousarray(np.asarray(metric_weights, dtype=np.float32))
    th = np.asarray(threshold, dtype=np.float32).reshape(1, 1)
    src_all = ei[0].astype(np.int64)
    dst_all = ei[1].astype(np.int64)
    in_maps = []
    perms = []
    ovfs = []
    for i in range(NCORES):
        seg = slice(i * EPC, (i + 1) * EPC)
        idxl, idxr, eas, ovf = _prepare_core(src_all[seg], dst_all[seg])
        perms.append(eas)
        ovfs.append(ovf)
        # rows [0:NSL] = half-0 slice, [NHPAD:NHPAD+NSL] = half-1 slice
        myl = np.ones((2 * NHPAD, D), dtype=np.float32)
        myr = np.ones((2 * NHPAD, D), dtype=np.float32)
        myl[:NSL] = lf[i * NSL:(i + 1) * NSL]
        myl[NHPAD:NHPAD + NSL] = lf[NHALF + i * NSL:NHALF + (i + 1) * NSL]
        myr[:NSL] = rf[i * NSL:(i + 1) * NSL]
        myr[NHPAD:NHPAD + NSL] = rf[NHALF + i * NSL:NHALF + (i + 1) * NSL]
        in_maps.append({
            "myl": myl, "myr": myr,
            "idxl": idxl, "idxr": idxr,
            "mw": mw, "th": th,
        })
    return in_maps, perms, ovfs


def run(inputs, trace=False, trace_kwargs=None):
    """Run on hardware; returns (output, BassKernelResults)."""
    from concourse.bass_utils import run_bass_kernel_spmd
    nc = _get_nc()
    in_maps, perms, ovfs = _prepare_in_maps(**inputs)
    res = run_bass_kernel_spmd(nc, in_maps, list(range(NCORES)), trace=trace,
                               **(trace_kwargs or {}))
    out = np.empty(E, dtype=np.float32)
    for i in range(NCORES):
        arr = res.results[i]["out"]          # [128, TOTG]
        sim_slot = np.asarray(arr).T.reshape(-1)   # slot s = g*128+p
        eas = perms[i]
        valid = eas >= 0
        out[i * EPC + eas[valid]] = sim_slot[valid]
        if len(ovfs[i]):
            eg = i * EPC + ovfs[i]
            out[eg] = _host_sims(inputs, eg)
    return out, res


def _host_sims(inputs, edge_ids):
    # reference-exact similarity for a handful of edges (overflow fallback)
    lf = np.asarray(inputs["left_features"], dtype=np.float32)
    rf = np.asarray(inputs["right_features"], dtype=np.float32)
    ei = np.asarray(inputs["edge_index"])
    mw = np.asarray(inputs["metric_weights"], dtype=np.float32)
    thv = 1.0 / (1.0 + np.exp(-float(np.asarray(inputs["threshold"]).ravel()[0])))
    lg = lf[ei[0][edge_ids]]
    rg = rf[ei[1][edge_ids]]
    s = np.zeros(len(edge_ids), dtype=np.float32)
    for h in range(H):
        a = lg * mw[h]
        b = rg * mw[h]
        dot = (a * b).sum(-1)
        na = np.maximum(np.sqrt((a * a).sum(-1)), EPS)
        nb = np.maximum(np.sqrt((b * b).sum(-1)), EPS)
        s += dot / (na * nb)
    s /= H
    return np.where(s < thv, 0.0, s).astype(np.float32)


def kernel(left_features, right_features, edge_index, metric_weights,
           threshold):
    inputs = dict(left_features=left_features,
                  right_features=right_features,
                  edge_index=edge_index,
                  metric_weights=metric_weights,
                  threshold=threshold)
    # a transient device fault can surface as NaNs (gather raced ahead of
    # its DMA); valid outputs are always finite (norms are eps-clamped),
    # so NaN unambiguously signals the fault -> retry up to 3 times
    for _attempt in range(4):
        out, _ = run(inputs)
        if not np.isnan(out).any():
            break
    return out


# revision 18
# speedup vs baseline: 1.2150x; 1.0398x over previous
"""Trainium2 Bass kernel for nn_AdaptiveGraphGenerator (gnn_message_passing).

Math: for each edge e = (s, t),
  sim[e] = mean_h cosine(l[s] * w_h, r[t] * w_h);  out[e] = sim if sim >= sigmoid(th) else 0.

Device algorithm (8 NeuronCores, SPMD, edges sharded 75000/core):
  1. Each core normalizes a slice of both feature tables into "hat" rows:
     hat[n] = concat_h( x[n]*w_h / (sqrt(2)*max(||x[n]*w_h||, eps)) )
     (256 bf16 values = 512 B per node). The tables are split into two
     25000-row halves per side; each core builds 3125 rows of each half and
     four AllGathers (L0, R0, L1, R1) assemble the halves in every core's
     HBM. Halving the tables (a) makes row indices fit int16 directly and
     (b) lets the edge phase start after the first two AllGathers, hiding
     the remaining collectives under gather work. The sqrt(2) folds the
     mean-over-heads 1/2 into the product of the two row norms.
  2. Per edge, sim = <hat_l[s], hat_r[t]> — one 256-long bf16 dot. Rows are
     fetched with dma_gather (int16 indices), spread round-robin over 4
     SWDGE queues so all 8 GpSimd Q7 cores generate descriptors in
     parallel. Edges are bucketed on the host by (s//25000, t//25000) in
     dependency order (0,0),(1,0),(0,1),(1,1).
  3. VectorE: product + pairwise-halving reduce (2x-mode tensor_tensor adds
     instead of the 1x-only tensor_reduce) + threshold; result DMA'd out.
Host does only index bookkeeping: half-bucketing (a permutation of each
core's edge shard) and the inverse permutation on the scalar outputs.
"""

import numpy as np

N, D, E, H = 50000, 128, 600000, 2
NCORES = 8
EPC = E // NCORES            # 75000 edges per core
K = 1024                     # rows per dma_gather call (ring capacity limit)
NBUCK = 4                    # (left half, right half) buckets
CAPB = 19 * K                # bucket capacity: mean 18750 + ~6 sigma
SLOTS = NBUCK * CAPB         # 77824 gather slots per core
TOTG = SLOTS // 128          # 608 output groups
BATCH = 2                    # gather calls per DVE compute chunk
NHALF = N // 2               # 25000 rows per table half
NSL = NHALF // NCORES        # 3125 rows built per core per half per side
NHPAD = 3200                 # padded to 25*128
EPS = 1e-8                   # torch cosine_similarity eps
SQRT2 = 1.4142135623730951
# bucket order = dependency order vs the AllGather sequence L0, R0, L1, R1
BUCKET_ORDER = [(0, 0), (1, 0), (0, 1), (1, 1)]

_CACHE = {}


def _build(d=D, capb=CAPB, k=K, batch=BATCH, nsl=NSL, nhpad=NHPAD,
           ncores=NCORES):
    from concourse import bass, bacc, mybir, tile
    from concourse.library_config import mlp

    f32 = mybir.dt.float32
    bf16 = mybir.dt.bfloat16
    i16 = mybir.dt.int16
    mult = mybir.AluOpType.mult
    add = mybir.AluOpType.add
    AF = mybir.ActivationFunctionType
    X = mybir.AxisListType.X

    slots = NBUCK * capb
    totg = slots // 128
    nblk = nhpad // 128                     # 25 blocks per sub-build
    calls_per_bucket = capb // k            # 19
    es = H * d                              # 256 elems (512 B bf16) per row
    nfull = nsl - nsl % 128                 # 3072 full-block rows
    nfblk = nfull // 128
    nrem = nsl - nfull                      # 53 ragged tail rows

    nc = bacc.Bacc("TRN2", target_bir_lowering=False, debug=False,
                   num_devices=ncores, num_swdge_queues=4)
    # per side: rows [0:nsl] = this core's half-0 slice, [nhpad:nhpad+nsl] =
    # half-1 slice (padding rows are 1.0)
    myl = nc.dram_tensor("myl", [2 * nhpad, d], f32, kind="ExternalInput").ap()
    myr = nc.dram_tensor("myr", [2 * nhpad, d], f32, kind="ExternalInput").ap()
    idxl = nc.dram_tensor("idxl", [128, slots // 16], i16,
                          kind="ExternalInput").ap()
    idxr = nc.dram_tensor("idxr", [128, slots // 16], i16,
                          kind="ExternalInput").ap()
    mw = nc.dram_tensor("mw", [H, d], f32, kind="ExternalInput").ap()
    th = nc.dram_tensor("th", [1, 1], f32, kind="ExternalInput").ap()
    out = nc.dram_tensor("out", [128, totg], f32, kind="ExternalOutput").ap()

    with tile.TileContext(nc) as tc:
        nc.gpsimd.load_library(mlp)
        with tc.tile_pool(name="const", bufs=1) as constp, \
             tc.tile_pool(name="dram", bufs=1, space="DRAM") as dramp:

            # ---- normalized half-table builds (blocked layout: partition p,
            # block b <-> slice row b*128+p), each followed by its AllGather.
            bld_ctx = tc.tile_pool(name="bld", bufs=1)
            bld = bld_ctx.__enter__()

            wrep = []
            for h in range(H):
                wrow = bld.tile([1, d], f32, name=f"wrow{h}", tag=f"wrow{h}")
                nc.sync.dma_start(out=wrow[:], in_=mw[h:h + 1, :])
                wf = bld.tile([128, d], f32, name=f"w2f{h}", tag=f"w2f{h}")
                nc.gpsimd.partition_broadcast(wf[:], wrow[:], 128)
                wfb = bld.tile([128, d], bf16, name=f"w2b{h}",
                               tag=f"w2b{h}")
                nc.vector.tensor_copy(out=wfb[:], in_=wf[:])
                wrep.append(wfb)

            # build order L0, R0, L1, R1 matches BUCKET_ORDER dependencies
            fulls = {}
            for side, half in ((0, 0), (1, 0), (0, 1), (1, 1)):
                my = myl if side == 0 else myr
                loc = dramp.tile([nsl, es], bf16, name=f"hatloc{side}{half}",
                                 tag=f"hatloc{side}{half}")
                ftf = bld.tile([128, nblk * d], f32, name="ftf", tag="ftf")
                nc.sync.dma_start(
                    out=ftf[:].rearrange("p (b d) -> p b d", d=d),
                    in_=my[half * nhpad:(half + 1) * nhpad, :]
                        .rearrange("(b p) d -> p b d", p=128))
                ft = bld.tile([128, nblk * d], bf16, name="ft", tag="ft")
                nc.vector.tensor_copy(out=ft[:], in_=ftf[:])
                us = []
                for h in range(H):
                    u = bld.tile([128, nblk * d], bf16, name=f"u{h}",
                                 tag=f"u{h}")
                    nc.vector.tensor_tensor(
                        out=u[:].rearrange("p (a b) -> p a b", b=d),
                        in0=ft[:].rearrange("p (a b) -> p a b", b=d),
                        in1=wrep[h][:].unsqueeze(1)
                            .to_broadcast([128, nblk, d]),
                        op=mult)
                    us.append(u)
                ss = bld.tile([128, H * nblk], f32, name="ss", tag="ss")
                usq = bld.tile([128, nblk * d], bf16, name="usq", tag="usq")
                for h in range(H):
                    nc.vector.tensor_tensor(out=usq[:], in0=us[h][:],
                                            in1=us[h][:], op=mult)
                    nc.vector.tensor_reduce(
                        out=ss[:, h * nblk:(h + 1) * nblk],
                        in_=usq[:].rearrange("p (a b) -> p a b", b=d),
                        axis=X, op=add)
                # inv = 1/max(sqrt(2*ss), sqrt(2)*eps)
                sq = bld.tile([128, H * nblk], f32, name="sq", tag="sq")
                nc.scalar.activation(out=sq[:], in_=ss[:], func=AF.Sqrt,
                                     scale=2.0)
                nc.vector.tensor_scalar_max(sq[:], sq[:], SQRT2 * EPS)
                inv = bld.tile([128, H * nblk], f32, name="inv", tag="inv")
                nc.vector.reciprocal(inv[:], sq[:])
                hat = bld.tile([128, nblk * es], bf16, name="hat", tag="hat")
                for h in range(H):
                    invexp = inv[:, h * nblk:(h + 1) * nblk].unsqueeze(2) \
                        .to_broadcast([128, nblk, d])
                    nc.vector.scalar_tensor_tensor(
                        out=hat[:].rearrange("p (a b) -> p a b", b=es)
                            [:, :, h * d:(h + 1) * d],
                        in0=us[h][:].rearrange("p (a b) -> p a b", b=d),
                        scalar=1.0, in1=invexp,
                        op0=mybir.AluOpType.bypass, op1=mult)
                # write rows [0:nsl] compactly (full blocks + ragged tail)
                nc.sync.dma_start(
                    out=loc[0:nfull, :]
                        .rearrange("(b p) d -> p b d", p=128),
                    in_=hat[:, 0:nfblk * es]
                        .rearrange("p (b d) -> p b d", d=es))
                if nrem:
                    nc.sync.dma_start(
                        out=loc[nfull:nsl, :],
                        in_=hat[0:nrem, nfblk * es:(nfblk + 1) * es])
                ful = dramp.tile([NHALF, es], bf16, name=f"hatfull{side}{half}",
                                 tag=f"hatfull{side}{half}",
                                 addr_space="Shared" if ncores > 4 else "Local")
                nc.gpsimd.collective_compute(
                    "AllGather", mybir.AluOpType.bypass,
                    replica_groups=[list(range(ncores))],
                    ins=[loc[:].opt()], outs=[ful[:].opt()])
                fulls[(side, half)] = ful
            bld_ctx.__exit__(None, None, None)

            # ---- edge-phase setup, deferred past the AllGather triggers so
            # nothing serializes ahead of the first table build
            idxl_sb = constp.tile([128, slots // 16], i16, name="idxl_sb")
            idxr_sb = constp.tile([128, slots // 16], i16, name="idxr_sb")
            nc.sync.dma_start(out=idxl_sb[:], in_=idxl[:])
            nc.sync.dma_start(out=idxr_sb[:], in_=idxr[:])
            out_sb = constp.tile([128, totg], f32, name="out_sb")
            tht = constp.tile([1, 1], f32, name="tht")
            nc.sync.dma_start(out=tht[:], in_=th[:])
            sig = constp.tile([1, 1], f32, name="sig")
            nc.scalar.activation(out=sig[:], in_=tht[:], func=AF.Sigmoid)
            thbc = constp.tile([128, 1], f32, name="thbc")
            nc.gpsimd.partition_broadcast(thbc[:], sig[:], 128)

            # ---- edge phase: half-bucketed gathers + fused dot + thresh
            gath_ctx = tc.tile_pool(name="gath", bufs=4)
            gath = gath_ctx.__enter__()
            work_ctx = tc.tile_pool(name="work", bufs=2)
            work = work_ctx.__enter__()
            ncalls = slots // k
            gpcall = k // 128                  # groups per call
            chunk_starts = list(range(0, ncalls, batch))
            PF = 6                             # l-gather prefetch depth
            lt_tiles = {}
            qctr = [0]                         # round-robin SWDGE queue picker

            def next_q():
                q = qctr[0] & 3
                qctr[0] += 1
                return q

            def issue_l(c0):
                nb = min(batch, ncalls - c0)
                lt = gath.tile([128, nb * gpcall * es], bf16, name="lt",
                               tag="lt", bufs=PF + 1)
                for j in range(nb):
                    ci = c0 + j
                    lh = BUCKET_ORDER[ci // calls_per_bucket][0]
                    isl = slice(ci * (k // 16), (ci + 1) * (k // 16))
                    dsl = slice(j * gpcall * es, (j + 1) * gpcall * es)
                    nc.gpsimd.dma_gather(
                        lt[:, dsl].rearrange("p (a b) -> p a b", b=es),
                        fulls[(0, lh)][:, :], idxl_sb[:, isl], k, k, es,
                        queue_num=next_q())
                lt_tiles[c0] = lt

            def process(c0):
                nb = min(batch, ncalls - c0)
                lt = lt_tiles.pop(c0)
                rt = gath.tile([128, nb * gpcall * es], bf16, name="rt",
                               tag="rt", bufs=3)
                for j in range(nb):
                    ci = c0 + j
                    rh = BUCKET_ORDER[ci // calls_per_bucket][1]
                    isl = slice(ci * (k // 16), (ci + 1) * (k // 16))
                    dsl = slice(j * gpcall * es, (j + 1) * gpcall * es)
                    nc.gpsimd.dma_gather(
                        rt[:, dsl].rearrange("p (a b) -> p a b", b=es),
                        fulls[(1, rh)][:, :], idxr_sb[:, isl], k, k, es,
                        queue_num=next_q())
                g = nb * gpcall                # groups this chunk
                # any fp8 operand forces tensor_tensor below 1x on DVE; cast
                # BOTH sides to bf16 on the ACT engine so the product runs in
                # 2x mode (DVE is the edge phase's pacing engine)
                ltb = work.tile([128, g * es], bf16, name="ltb", tag="ltb")
                rtb = work.tile([128, g * es], bf16, name="rtb", tag="rtb")
                nc.scalar.activation(out=ltb[:], in_=lt[:], func=AF.Copy)
                nc.scalar.activation(out=rtb[:], in_=rt[:], func=AF.Copy)
                prod = work.tile([128, g * es], bf16, name="prod", tag="prod")
                nc.vector.tensor_tensor(out=prod[:], in0=ltb[:], in1=rtb[:],
                                        op=mult)
                # pairwise-halving reduce: contiguous halves keep the DVE in
                # 2x mode (tensor_reduce only has a 1x uop)
                w = es
                src = prod
                while w > 16:
                    w //= 2
                    ht = work.tile([128, g * w], bf16, name=f"h{w}",
                                   tag=f"h{w}")
                    s3 = src[:].rearrange("p (a b) -> p a b", b=2 * w)
                    nc.vector.tensor_tensor(
                        out=ht[:].rearrange("p (a b) -> p a b", b=w),
                        in0=s3[:, :, 0:w], in1=s3[:, :, w:2 * w], op=add)
                    src = ht
                sim = work.tile([128, g], f32, name="sim", tag="sim")
                nc.vector.tensor_reduce(
                    out=sim[:],
                    in_=src[:].rearrange("p (a b) -> p a b", b=16),
                    axis=X, op=add)
                nc.vector.scalar_tensor_tensor(
                    out=out_sb[:, c0 * gpcall:(c0 + nb) * gpcall],
                    in0=sim[:], scalar=thbc[:, 0:1], in1=sim[:],
                    op0=mybir.AluOpType.is_ge, op1=mult)

            nchunk = len(chunk_starts)
            qsize = max(1, nchunk // 4)
            done_g = 0
            for i, c0 in enumerate(chunk_starts):
                issue_l(c0)
                if i >= PF:
                    process(chunk_starts[i - PF])
                    # stream completed quarters of the output early
                    pc = i - PF + 1
                    if pc % qsize == 0 and pc < nchunk:
                        g1 = chunk_starts[pc - 1] * gpcall + \
                            min(batch, ncalls - chunk_starts[pc - 1]) * gpcall
                        if g1 > done_g:
                            nc.sync.dma_start(out=out[:, done_g:g1],
                                              in_=out_sb[:, done_g:g1])
                            done_g = g1
            for c0 in chunk_starts[-PF:] if PF < nchunk else chunk_starts:
                process(c0)
            nc.sync.dma_start(out=out[:, done_g:totg],
                              in_=out_sb[:, done_g:totg])
            work_ctx.__exit__(None, None, None)
            gath_ctx.__exit__(None, None, None)

    nc.compile()
    return nc


def _get_nc():
    if "nc" not in _CACHE:
        _CACHE["nc"] = _build()
    return _CACHE["nc"]


def _wrap16(idx):
    """[slots] int -> [128, slots//16] int16 (wrap-16, replicated 8x)."""
    blk = idx.reshape(-1, 16).T.astype(np.int16)
    return np.ascontiguousarray(np.tile(blk, (8, 1)))


def _prepare_core(src, dst, capb=CAPB):
    """Bucket one core's edges by (src half, dst half) in BUCKET_ORDER.
    Returns idxl, idxr (wrap16 arrays) and edge_at_slot (int64, -1 pad)."""
    slots = NBUCK * capb
    rl = src.astype(np.int64)
    rr = dst.astype(np.int64)
    lh = rl // NHALF
    rh = rr // NHALF
    idxl = np.zeros(slots, dtype=np.int64)
    idxr = np.zeros(slots, dtype=np.int64)
    edge_at_slot = np.full(slots, -1, dtype=np.int64)
    overflow = []
    for b, (blh, brh) in enumerate(BUCKET_ORDER):
        ids = np.nonzero((lh == blh) & (rh == brh))[0]
        if len(ids) > capb:
            # vanishingly rare (capacity is mean + ~6 sigma): excess edges
            # fall back to a host-side computation in run()
            overflow.append(ids[capb:])
            ids = ids[:capb]
        # ascending left rows per bucket -> monotonic gather addresses
        ids = ids[np.argsort(rl[ids], kind="stable")]
        base = b * capb
        edge_at_slot[base:base + len(ids)] = ids
        idxl[base:base + len(ids)] = rl[ids] % NHALF
        idxr[base:base + len(ids)] = rr[ids] % NHALF
        # padding slots keep index 0 (valid row of every half-table)
    ovf = np.concatenate(overflow) if overflow else np.empty(0, dtype=np.int64)
    return _wrap16(idxl), _wrap16(idxr), edge_at_slot, ovf


def _prepare_in_maps(left_features, right_features, edge_index,
                     metric_weights, threshold):
    lf = np.ascontiguousarray(np.asarray(left_features, dtype=np.float32))
    rf = np.ascontiguousarray(np.asarray(right_features, dtype=np.float32))
    ei = np.asarray(edge_index)
    mw = np.ascontiguousarray(np.asarray(metric_weights, dtype=np.float32))
    th = np.asarray(threshold, dtype=np.float32).reshape(1, 1)
    src_all = ei[0].astype(np.int64)
    dst_all = ei[1].astype(np.int64)
    in_maps = []
    perms = []
    ovfs = []
    for i in range(NCORES):
        seg = slice(i * EPC, (i + 1) * EPC)
        idxl, idxr, eas, ovf = _prepare_core(src_all[seg], dst_all[seg])
        perms.append(eas)
        ovfs.append(ovf)
        # rows [0:NSL] = half-0 slice, [NHPAD:NHPAD+NSL] = half-1 slice
        myl = np.ones((2 * NHPAD, D), dtype=np.float32)
        myr = np.ones((2 * NHPAD, D), dtype=np.float32)
        myl[:NSL] = lf[i * NSL:(i + 1) * NSL]
        myl[NHPAD:NHPAD + NSL] = lf[NHALF + i * NSL:NHALF + (i + 1) * NSL]
        myr[:NSL] = rf[i * NSL:(i + 1) * NSL]
        myr[NHPAD:NHPAD + NSL] = rf[NHALF + i * NSL:NHALF + (i + 1) * NSL]
        in_maps.append({
            "myl": myl, "myr": myr,
            "idxl": idxl, "idxr": idxr,
            "mw": mw, "th": th,
        })
    return in_maps, perms, ovfs


def run(inputs, trace=False, trace_kwargs=None):
    """Run on hardware; returns (output, BassKernelResults)."""
    from concourse.bass_utils import run_bass_kernel_spmd
    nc = _get_nc()
    in_maps, perms, ovfs = _prepare_in_maps(**inputs)
    res = run_bass_kernel_spmd(nc, in_maps, list(range(NCORES)), trace=trace,
                               **(trace_kwargs or {}))
    out = np.empty(E, dtype=np.float32)
    for i in range(NCORES):
        arr = res.results[i]["out"]          # [128, TOTG]
        sim_slot = np.asarray(arr).T.reshape(-1)   # slot s = g*128+p
        eas = perms[i]
        valid = eas >= 0
        out[i * EPC + eas[valid]] = sim_slot[valid]
        if len(ovfs[i]):
            eg = i * EPC + ovfs[i]
            out[eg] = _host_sims(inputs, eg)
    return out, res


def _host_sims(inputs, edge_ids):
    # reference-exact similarity for a handful of edges (overflow fallback)
    lf = np.asarray(inputs["left_features"], dtype=np.float32)
    rf = np.asarray(inputs["right_features"], dtype=np.float32)
    ei = np.asarray(inputs["edge_index"])
    mw = np.asarray(inputs["metric_weights"], dtype=np.float32)
    thv = 1.0 / (1.0 + np.exp(-float(np.asarray(inputs["threshold"]).ravel()[0])))
    lg = lf[ei[0][edge_ids]]
    rg = rf[ei[1][edge_ids]]
    s = np.zeros(len(edge_ids), dtype=np.float32)
    for h in range(H):
        a = lg * mw[h]
        b = rg * mw[h]
        dot = (a * b).sum(-1)
        na = np.maximum(np.sqrt((a * a).sum(-1)), EPS)
        nb = np.maximum(np.sqrt((b * b).sum(-1)), EPS)
        s += dot / (na * nb)
    s /= H
    return np.where(s < thv, 0.0, s).astype(np.float32)


def kernel(left_features, right_features, edge_index, metric_weights,
           threshold):
    inputs = dict(left_features=left_features,
                  right_features=right_features,
                  edge_index=edge_index,
                  metric_weights=metric_weights,
                  threshold=threshold)
    # a transient device fault can surface as NaNs (gather raced ahead of
    # its DMA); valid outputs are always finite (norms are eps-clamped),
    # so NaN unambiguously signals the fault -> retry up to 3 times
    for _attempt in range(4):
        out, _ = run(inputs)
        if not np.isnan(out).any():
            break
    return out
